# revision 30
# baseline (speedup 1.0000x reference)
"""Trainium2 Bass kernel for nn_OA_Layer (offset-attention layer).

Reference (per batch b, C=256, N=4096, CQK=64):
    xs = x + xyz
    q = k = wqk @ xs + bqk          [64, N]
    v = wv @ xs + bv                [C, N]
    E = q^T q                       [N, N]  (symmetric, since q == k)
    attn = softmax(E, rows) ; attn /= (1e-9 + attn.sum(rows))
    x_r = v @ attn
    t = wt @ (xs - x_r) ; t = BN(t) ; x_r = leaky_relu(t, 0.2)
    out = xs + x_r

Sharding: data-parallel over batch B=8 across 8 cores (1 batch/core).

Implementation (v3):
  - fp16 attention core (1 PE cycle/row), fp32 PSUM accumulation.
  - exp shift -d[n] (d = ||q_n||^2; cancels exactly in softmax+renorm)
    folded into the E matmul as a 65th contraction row.
  - q bias folded into the q matmul as a K=1 rank-1 update.
  - pass 1 computes rowsums via fused Exp+accum on 2048-wide strips; the
    exp tiles of the first S_STORE row-blocks are kept in SBUF so pass 2
    skips their E+exp recompute entirely.
  - colsum'[m] via matmuls with the 1-column invrs vector stationary.
  - x_r = (v' @ a) * invcs[m]; bv folded into bt' = bt - wt @ bv on host.
  - BN+bias folded to t*g + bp_eff on host.
  - pass 2: per j, stored blocks first (dense matmul streak keeps the PE
    clock warm), then software-pipelined E one block ahead; the per-j
    tail (colsum renorm + output projection) is emitted split across the
    next j's iterations so the PE queue never blocks on DVE latency.
"""

import numpy as np

import concourse.bass as bass
import concourse.tile as tile
from concourse import bacc, mybir
from concourse._compat import with_exitstack

F32 = mybir.dt.float32
F16 = mybir.dt.float16
BF16 = mybir.dt.bfloat16

C = 256
CQK = 64
P = 128
KQ = CQK + 1   # 64 q rows + 1 bias row
S_STORE = 12   # row-blocks whose exp tiles stay resident in SBUF (mult of 4)
D0 = 40.0      # shift for the w~ = exp((d-D0)/2) triangle colsum weights
BN_EPS = 1e-5


def build_kernel(N=4096, debug=False):
    nc = bacc.Bacc("TRN2", target_bir_lowering=False, debug=debug,
                   num_devices=8)

    x_d = nc.declare_dram_parameter("x", [C, N], F32, isOutput=False)
    xyz_d = nc.declare_dram_parameter("xyz", [C, N], F32, isOutput=False)
    wqkT_d = nc.declare_dram_parameter("wqkT", [C, CQK], F16, isOutput=False)
    wvT_d = nc.declare_dram_parameter("wvT", [C, C], F16, isOutput=False)
    wtT_d = nc.declare_dram_parameter("wtT", [C, C], F16, isOutput=False)
    bqk_d = nc.declare_dram_parameter("bqk", [CQK, 1], F32, isOutput=False)
    g_d = nc.declare_dram_parameter("g", [C, 1], F32, isOutput=False)
    bp_d = nc.declare_dram_parameter("bp", [C, 1], F32, isOutput=False)
    out_d = nc.declare_dram_parameter("out", [C, N], F32, isOutput=True)

    with tile.TileContext(nc) as tc:
        _emit(nc, tc, N,
              x_d, xyz_d, wqkT_d, wvT_d, wtT_d, bqk_d, g_d, bp_d, out_d)
    nc.compile()
    return nc


@with_exitstack
def _emit(ctx, nc, tc, N,
          x_d, xyz_d, wqkT_d, wvT_d, wtT_d, bqk_d, g_d, bp_d, out_d):
    NB = N // P          # 32 row-blocks of 128
    MC = N // 512        # 8 column chunks of 512
    ek = ctx.enter_context

    consts = ek(tc.tile_pool(name="consts", bufs=1))
    big = ek(tc.tile_pool(name="big", bufs=1))
    stats = ek(tc.tile_pool(name="stats", bufs=1))

    # ---- constant / resident tensors ----
    wqkT = consts.tile([P, 2 * CQK], F16)
    nc.sync.dma_start(wqkT[:].rearrange("p (t m) -> p t m", t=2),
                      wqkT_d[:].rearrange("(t p) m -> p t m", p=P))
    wvT = consts.tile([P, 2 * C], F16)
    nc.sync.dma_start(wvT[:].rearrange("p (t m) -> p t m", t=2),
                      wvT_d[:].rearrange("(t p) m -> p t m", p=P))
    wtT = consts.tile([P, 2 * C], F16)
    nc.sync.dma_start(wtT[:].rearrange("p (t m) -> p t m", t=2),
                      wtT_d[:].rearrange("(t p) m -> p t m", p=P))
    bqk_row32 = consts.tile([1, CQK], F32)
    nc.sync.dma_start(bqk_row32[:], bqk_d[:])     # [64,1] dram -> [1,64]
    bqk_row = consts.tile([1, CQK], F16)
    nc.vector.tensor_copy(bqk_row[:], bqk_row32[:])
    # NOTE: keep every SBUF tile's per-partition footprint a multiple of
    # 64 B — a 32 B-aligned destination costs ~20% on wide ACT writes and
    # ~75 ns per matmul SBUF stream (measured v2 vs v3).
    g_t = consts.tile([P, 16], F32)
    bp_t = consts.tile([P, 16], F32)
    for h in range(2):
        nc.sync.dma_start(g_t[:, h:h + 1], g_d[h * P:(h + 1) * P, :])
        nc.sync.dma_start(bp_t[:, h:h + 1], bp_d[h * P:(h + 1) * P, :])
    ones64 = consts.tile([CQK, 32], F16)
    nc.vector.memset(ones64[:], 1.0)
    ones_row = consts.tile([1, P], F16)
    nc.vector.memset(ones_row[:], 1.0)
    ones512 = consts.tile([1, 512], F16)
    nc.vector.memset(ones512[:], 1.0)

    xs = big.tile([P, 2 * N], F32)
    qdL = big.tile([KQ, N], F16)   # rows 0-63 q, row 64 = -d/2 (weights side)
    qR = big.tile([KQ, N], F16)    # rows 0-63 q, row 64 = ones (moving side)
    vT = big.tile([P, NB * C], BF16)
    rs_acc = stats.tile([P, 4 * NB], F32)
    rs_sum = stats.tile([P, NB], F32)
    invrs32 = stats.tile([P, NB], F32)
    invrsb = stats.tile([P, NB], BF16)
    dcol = stats.tile([P, NB], F32)      # d[n] per-block columns
    wcol = stats.tile([P, NB], BF16)     # w~ = exp((d-D0)/2)
    resc32 = stats.tile([P, NB], F32)    # exp((D0-d)/2)
    cs1col = stats.tile([P, NB], F32)    # transposed colsum parts
    # CS1row[m - 2*512] for chunks 2..7: transposed rowsum contributions
    CS1row = stats.tile([1, (MC - S_STORE // 4) * 512], F32)
    nc.vector.memset(CS1row[:], 0.0)

    nc.vector.memset(qR[CQK:KQ, :], 1.0)

    # ================= setup (scoped; xsh freed afterwards) ===============
    with (
        tc.tile_pool(name="setp", bufs=1) as setp,
        tc.tile_pool(name="zpool", bufs=2) as zpool,
        tc.tile_pool(name="qvps", bufs=2, space=bass.MemorySpace.PSUM) as qvps,
        tc.tile_pool(name="sqp", bufs=2) as sqp,
    ):
        xsh = setp.tile([P, 2 * N], F16)
        CH = 2048
        for pch in range(N // CH):
            c0 = pch * CH
            for h in range(2):
                for cq in range(0, CH, 1024):   # finer DMAs -> more queues
                    nc.sync.dma_start(
                        xs[:, h * N + c0 + cq: h * N + c0 + cq + 1024],
                        x_d[h * P:(h + 1) * P, c0 + cq:c0 + cq + 1024])
                zin = zpool.tile([P, CH], F32, tag="zin")
                for cq in range(0, CH, 1024):
                    nc.sync.dma_start(
                        zin[:, cq:cq + 1024],
                        xyz_d[h * P:(h + 1) * P, c0 + cq:c0 + cq + 1024])
                sl = xs[:, h * N + c0: h * N + c0 + CH]
                nc.vector.tensor_add(sl, sl, zin[:])
                nc.scalar.copy(xsh[:, h * N + c0: h * N + c0 + CH], sl)
            for jc in range(pch * 4, pch * 4 + 4):
                q_ps = qvps.tile([CQK, 512], F32, tag="q_ps")
                for k in range(2):
                    nc.tensor.matmul(
                        q_ps[:], wqkT[:, k * CQK:(k + 1) * CQK],
                        xsh[:, k * N + jc * 512: k * N + jc * 512 + 512],
                        start=(k == 0), stop=False)
                nc.tensor.matmul(q_ps[:], bqk_row[:], ones512[:],
                                 start=False, stop=True)
                nc.scalar.copy(qdL[0:CQK, jc * 512:(jc + 1) * 512], q_ps[:])
                nc.vector.tensor_copy(qR[0:CQK, jc * 512:(jc + 1) * 512],
                                      q_ps[:])
                sq = sqp.tile([CQK, 512], F16, tag="sq")
                qs = qdL[0:CQK, jc * 512:(jc + 1) * 512]
                nc.vector.tensor_mul(sq[:], qs, qs)
                dg_ps = qvps.tile([1, 512], F32, tag="dg")
                nc.tensor.matmul(dg_ps[:], ones64[:, 0:1], sq[:],
                                 start=True, stop=True)
                nc.vector.tensor_scalar_mul(
                    qdL[CQK:KQ, jc * 512:(jc + 1) * 512], dg_ps[:], -0.5)
                drow = sqp.tile([1, 512], F32, tag="drow")
                nc.vector.tensor_copy(drow[:], dg_ps[:])
                for bq in range(4):
                    blk = jc * 4 + bq
                    nc.sync.dma_start(dcol[:, blk:blk + 1],
                                      drow[0:1, bq * P:(bq + 1) * P])
        nc.vector.tensor_scalar_add(dcol[:], dcol[:], -D0)
        nc.scalar.activation(wcol[:], dcol[:],
                             mybir.ActivationFunctionType.Exp, scale=0.5)
        nc.scalar.activation(resc32[:], dcol[:],
                             mybir.ActivationFunctionType.Exp, scale=-0.5)
        # dense v-matmul burst (also warms the PE clock before pass 1)
        for i2 in range(NB // 4):
            v_ps = qvps.tile([P, 4 * C], F32, tag="v_ps")
            for b in range(4):
                i = i2 * 4 + b
                for k in range(2):
                    nc.tensor.matmul(v_ps[:, b * C:(b + 1) * C],
                                     xsh[:, k * N + i * P: k * N + i * P + P],
                                     wvT[:, k * C:(k + 1) * C],
                                     start=(k == 0), stop=(k == 1))
            nc.vector.tensor_copy(vT[:, i2 * 4 * C:(i2 + 1) * 4 * C], v_ps[:])

    # resident exp tiles for the first S_STORE row-blocks (after xsh freed)
    astp = ek(tc.tile_pool(name="astp", bufs=1))
    astore = astp.tile([P, S_STORE * N], BF16)

    # ====== pass 1 (triangle): weighted rowsums of exp(E - d/2) ==========
    # Stored blocks compute full strips; blocks >= S_STORE only compute
    # chunks w >= their own unit u (E is symmetric).  The missing left
    # parts come from w~-weighted colsums of the strictly-upper tiles,
    # rescaled by exp((D0-d_m)/2) at combine time.
    SU = S_STORE // 4            # first triangle unit
    with (
        tc.tile_pool(name="p1ps", bufs=2, space=bass.MemorySpace.PSUM) as p1ps,
        tc.tile_pool(name="c1ps", bufs=2, space=bass.MemorySpace.PSUM) as c1ps,
        tc.tile_pool(name="p1sc", bufs=2) as p1sc,
    ):
        pend_cs1 = []

        def flush_cs1():
            # cs1 matmuls depend on the previous piece's ACT output; they
            # are deferred until after the NEXT piece's E matmuls so the
            # in-order PE queue never stalls waiting for the ACT.
            while pend_cs1:
                dst_sl, iw, w = pend_cs1.pop(0)
                cs1 = c1ps.tile([1, 512], F32, tag="cs1",
                                name=f"cs1_{iw}_{w}")
                nc.tensor.matmul(cs1[:], wcol[:, iw:iw + 1], dst_sl,
                                 start=True, stop=True)
                sl = CS1row[:, (w - SU) * 512:(w - SU + 1) * 512]
                nc.vector.tensor_add(sl, sl, cs1[:])

        for i in range(NB):
            u = i // 4
            c0 = 0 if i < S_STORE else u * 512
            col = c0
            pidx = 0
            while col < N:
                pw = min(1536, N - col)
                estrip = p1ps.tile([P, 1536], F32, tag="estrip",
                                   name=f"es_{i}_{pidx}")
                for ch in range(pw // 512):
                    nc.tensor.matmul(
                        estrip[:, ch * 512:(ch + 1) * 512],
                        qdL[0:KQ, i * P:(i + 1) * P],
                        qR[0:KQ, col + ch * 512: col + (ch + 1) * 512],
                        start=True, stop=True)
                flush_cs1()
                if i < S_STORE:
                    dst = astore[:, i * N + col: i * N + col + pw]
                else:
                    sink = p1sc.tile([P, 1536], BF16, tag="sink", name="sink")
                    dst = sink[:, 0:pw]
                nc.scalar.activation(
                    dst, estrip[:, 0:pw], mybir.ActivationFunctionType.Exp,
                    accum_out=rs_acc[:, 4 * i + pidx: 4 * i + pidx + 1])
                for ch in range(pw // 512):
                    w = (col + ch * 512) // 512
                    if w >= SU and w > u:
                        pend_cs1.append(
                            (dst[:, ch * 512:(ch + 1) * 512], i, w))
                col += pw
                pidx += 1
            if pidx == 1:
                nc.vector.tensor_copy(rs_sum[:, i:i + 1],
                                      rs_acc[:, 4 * i: 4 * i + 1])
            else:
                acc = rs_acc[:, 4 * i: 4 * i + 1]
                for pk in range(1, pidx):
                    nc.vector.tensor_add(rs_sum[:, i:i + 1], acc,
                                         rs_acc[:, 4 * i + pk: 4 * i + pk + 1])
                    acc = rs_sum[:, i:i + 1]
            if i >= S_STORE:
                # add transposed contribution: rs += resc * CS1[block cols]
                nc.sync.dma_start(cs1col[:, i:i + 1],
                                  CS1row[0:1, i * P - SU * 512:
                                         (i + 1) * P - SU * 512])
                nc.vector.tensor_mul(cs1col[:, i:i + 1], cs1col[:, i:i + 1],
                                     resc32[:, i:i + 1])
                nc.vector.tensor_add(rs_sum[:, i:i + 1], rs_sum[:, i:i + 1],
                                     cs1col[:, i:i + 1])
            nc.vector.reciprocal(invrs32[:, i:i + 1], rs_sum[:, i:i + 1])
            nc.vector.tensor_copy(invrsb[:, i:i + 1], invrs32[:, i:i + 1])
            nc.vector.tensor_scalar_mul(vT[:, i * C:(i + 1) * C],
                                        vT[:, i * C:(i + 1) * C],
                                        invrs32[:, i:i + 1])
        flush_cs1()

    # ================= pass 2 =============================================
    with (
        tc.tile_pool(name="e2ps", bufs=3, space=bass.MemorySpace.PSUM) as e2ps,
        tc.tile_pool(name="xrps", bufs=2, space=bass.MemorySpace.PSUM) as xrps,
        tc.tile_pool(name="csps", bufs=1, space=bass.MemorySpace.PSUM) as csps,
        tc.tile_pool(name="a2p", bufs=3) as a2p,
        tc.tile_pool(name="tails", bufs=1) as tails,
    ):
        e2_t = {}
        xr_t = {}
        cs_t = {}
        tail_st = {}

        def emit_E(j, i):
            e2 = e2ps.tile([P, 512], F32, tag="e2", name=f"e2_{j}_{i}")
            nc.tensor.matmul(e2[:], qdL[0:KQ, i * P:(i + 1) * P],
                             qR[0:KQ, j * 512:(j + 1) * 512],
                             start=True, stop=True)
            e2_t[(j, i)] = e2

        def tail_prep(j):
            cs = cs_t.pop(j)
            cs_eps = tails.tile([1, 512], F32, tag="cs_eps")
            nc.vector.tensor_scalar_add(cs_eps[:], cs[:], 1e-9)
            csr32 = tails.tile([1, 512], F32, tag="csr32")
            nc.vector.reciprocal(csr32[:], cs_eps[:])
            csr16 = tails.tile([1, 512], F16, tag="csr16")
            nc.vector.tensor_copy(csr16[:], csr32[:])
            tail_st[j] = csr16

        def tail_mid(j):
            csr16 = tail_st.pop(j)
            xr = xr_t[j]
            bc_ps = e2ps.tile([P, 512], F32, tag="e2", name=f"bc_{j}")
            nc.tensor.matmul(bc_ps[:], ones_row[:], csr16[:],
                             start=True, stop=True)
            invcs_bc = tails.tile([P, 512], F32, tag="invcs_bc")
            nc.vector.tensor_copy(invcs_bc[:], bc_ps[:])
            ys = []
            for h in range(2):
                tmp = tails.tile([P, 512], F32, tag=f"tmp{h}")
                nc.vector.tensor_mul(tmp[:], xr[h][:], invcs_bc[:])
                y_h = tails.tile([P, 512], F16, tag=f"y{h}")
                nc.vector.tensor_sub(
                    y_h[:], xs[:, h * N + j * 512: h * N + j * 512 + 512],
                    tmp[:])
                ys.append(y_h)
            tail_st[j] = ys

        def tail_fire(j):
            ys = tail_st.pop(j)
            xr = xr_t.pop(j)
            for ho in range(2):
                t_ps = xr[ho]   # reuse the drained x_r PSUM tile
                for k in range(2):
                    nc.tensor.matmul(
                        t_ps[:], wtT[:, k * C + ho * P: k * C + ho * P + P],
                        ys[k][:], start=(k == 0), stop=(k == 1))
                bn = tails.tile([P, 512], F32, tag=f"bn{ho}")
                nc.vector.tensor_scalar(bn[:], t_ps[:], g_t[:, ho:ho + 1],
                                        bp_t[:, ho:ho + 1],
                                        mybir.AluOpType.mult,
                                        mybir.AluOpType.add)
                lr = tails.tile([P, 512], F32, tag=f"lr{ho}")
                nc.vector.scalar_tensor_tensor(lr[:], bn[:], 0.2, bn[:],
                                               mybir.AluOpType.mult,
                                               mybir.AluOpType.max)
                o_t = tails.tile([P, 512], F32, tag=f"o{ho}")
                nc.vector.tensor_add(
                    o_t[:], lr[:],
                    xs[:, ho * N + j * 512: ho * N + j * 512 + 512])
                nc.sync.dma_start(
                    out_d[ho * P:(ho + 1) * P, j * 512:(j + 1) * 512], o_t[:])

        def next_live(j, i):
            """next (j', i') with i' >= S_STORE in scan order, else None."""
            i += 1
            while True:
                if i >= NB:
                    j, i = j + 1, S_STORE
                    if j >= MC:
                        return None
                if i >= S_STORE:
                    return (j, i)
                i += 1

        for j in range(MC):
            if j > 0:
                tail_prep(j - 1)
            xr_t[j] = [xrps.tile([P, 512], F32, tag=f"xr{h}", name=f"xr{h}_{j}")
                       for h in range(2)]
            cs_t[j] = csps.tile([1, 512], F32, tag="cs", name=f"cs_{j}")
            if j == 0:
                emit_E(0, S_STORE)
            for i in range(NB):
                if i >= S_STORE:
                    nxt = next_live(j, i)
                    if nxt:
                        emit_E(*nxt)
                    e2 = e2_t.pop((j, i))
                    a2t = a2p.tile([P, 512], BF16, tag="a2")
                    nc.scalar.activation(a2t[:], e2[:],
                                         mybir.ActivationFunctionType.Exp)
                    a2 = a2t[:]
                else:
                    a2 = astore[:, i * N + j * 512: i * N + j * 512 + 512]
                first, last = (i == 0), (i == NB - 1)
                for h in range(2):
                    nc.tensor.matmul(
                        xr_t[j][h][:],
                        vT[:, i * C + h * P: i * C + h * P + P],
                        a2, start=first, stop=last)
                nc.tensor.matmul(cs_t[j][:], invrsb[:, i:i + 1], a2,
                                 start=first, stop=last,
                                 skip_group_check=True)
                if j > 0:
                    if i == 3:
                        tail_mid(j - 1)
                    elif i == 5:
                        tail_fire(j - 1)
        tail_prep(MC - 1)
        tail_mid(MC - 1)
        tail_fire(MC - 1)


# ---------------------------------------------------------------------------
# host-side wrapper
# ---------------------------------------------------------------------------
_NC_CACHE = {}


def _get_nc(N=4096):
    if N not in _NC_CACHE:
        _NC_CACHE[N] = build_kernel(N=N)
    return _NC_CACHE[N]


def host_prep(wqk, bqk, wv, bv, wt, bt, bn_gamma, bn_beta, bn_mean, bn_var):
    wqk = np.asarray(wqk, np.float32)
    wv = np.asarray(wv, np.float32)
    wt = np.asarray(wt, np.float32)
    g = (np.asarray(bn_gamma, np.float32)
         / np.sqrt(np.asarray(bn_var, np.float32) + BN_EPS))
    bp = np.asarray(bn_beta, np.float32) - np.asarray(bn_mean, np.float32) * g
    btp = np.asarray(bt, np.float32) - wt @ np.asarray(bv, np.float32)
    bp_eff = btp * g + bp
    return {
        "wqkT": np.ascontiguousarray(wqk.T).astype(np.float16),
        "wvT": np.ascontiguousarray(wv.T).astype(np.float16),
        "wtT": np.ascontiguousarray(wt.T).astype(np.float16),
        "bqk": np.asarray(bqk, np.float32).reshape(CQK, 1),
        "g": g.reshape(C, 1),
        "bp": bp_eff.reshape(C, 1),
    }


def kernel(x, xyz, wqk, bqk, wv, bv, wt, bt, bn_gamma, bn_beta, bn_mean,
           bn_var, _profile=False):
    from concourse.bass_utils import run_bass_kernel_spmd

    x = np.asarray(x, np.float32)
    xyz = np.asarray(xyz, np.float32)
    B, Cc, N = x.shape
    assert Cc == C and B == 8
    nc = _get_nc(N)
    wmap = host_prep(wqk, bqk, wv, bv, wt, bt, bn_gamma, bn_beta, bn_mean,
                     bn_var)
    in_maps = [
        {"x": np.ascontiguousarray(x[b]),
         "xyz": np.ascontiguousarray(xyz[b]), **wmap}
        for b in range(B)
    ]
    res = run_bass_kernel_spmd(nc, in_maps, list(range(8)), trace=_profile)
    out = np.stack([res.results[b]["out"] for b in range(B)], axis=0)
    if _profile:
        return out, res
    return out
